# revision 1
# baseline (speedup 1.0000x reference)
"""Causal attention + output projection on 8 Trainium2 NeuronCores, v3.

Problem (hardcoded): B=2, H=12, T=2048, D=64, DIM=768, fp32.

Sharding: 24 (b, h) pairs -> 3 heads per core; cores 0-3 take b=0,
cores 4-7 take b=1.  Each core computes attention for its 3 heads plus
the partial output projection; the host sums the 4 bf16 partials per
batch in fp32.  No collectives.

All matmuls run in bf16 (1 PE cycle/row vs 4 for the fp32 baseline).
The additive attention bias is applied MULTIPLICATIVELY: the host ships
eb = exp(bias) (0 where causally masked) in bf16 and the device computes
P = exp(qk) * eb with one DVE multiply per region (2-byte operands hit
the DVE 2x mode) -- this needs no extra PE work and gives exact masking
for free.  Layout is fully transposed ([s, q]) so no on-chip transposes
are needed; softmax denominators come from 64 ones-columns baked into
va (they ride along the PV matmul at zero cost, since matmul time only
depends on the output free size).

Scheduling: the three heads' region streams are interleaved round-robin
so the in-order PE/Act/DVE queues always hold an independent ready
instruction; PV lags its region by one slot (software pipelining); psl
is a 2-deep [128,1024] PSUM ring; exp is per-region on Act, trimmed on
the diagonal; every 4th eb-multiply runs on GPSIMD for balance.  The
projection stacks heads 0+1 on 128 partitions, rotates its quarter-bank
accumulators over the three idle psy banks + psp, and its PSUM->SBUF
copies run on DVE (GPSIMD cannot touch PSUM on real hardware).
Startup DMAs are spread across the SP/Act/Pool DGE queues (each DMA
costs ~1.6us of serialized setup on one queue).
"""

import math

import numpy as np
import ml_dtypes

B, H, T, D = 2, 12, 2048, 64
DIM = H * D
NCORES = 8
HPC = 3            # heads per core
P = 128
QC = 512           # q-chunk width
NJ = T // QC       # 4 q-chunks
NT = T // P        # 16 s-tiles

_PROGRAM = None


def _c0(i, j):
    return max(0, P * i - QC * j)


def _eb_cols(j):
    return sum(QC - _c0(i, j) for i in range(4 * j + 4))


EB_OFF = [0]
for _j in range(NJ):
    EB_OFF.append(EB_OFF[-1] + _eb_cols(_j))
EB_TOT = EB_OFF[-1]  # 17408 per head


def _build_program():
    import concourse.bass as bass
    import concourse.mybir as mybir
    import concourse.tile as tile
    from concourse import bacc
    from contextlib import ExitStack

    dt = mybir.dt
    f32 = dt.float32
    bf16 = dt.bfloat16
    EXP = mybir.ActivationFunctionType.Exp
    ds = bass.ds

    nc = bacc.Bacc("TRN2", num_devices=NCORES)
    qk_d = nc.declare_dram_parameter("qk", [D, HPC * 2 * T], bf16, isOutput=False)
    va_d = nc.declare_dram_parameter("va", [P, HPC * T], bf16, isOutput=False)
    eb_d = nc.declare_dram_parameter("eb", [P, HPC * EB_TOT], bf16, isOutput=False)
    w01_d = nc.declare_dram_parameter("w01", [P, DIM], bf16, isOutput=False)
    w2_d = nc.declare_dram_parameter("w2", [D, DIM], bf16, isOutput=False)
    out_d = nc.declare_dram_parameter("out", [T, DIM], bf16, isOutput=True)

    with tile.TileContext(nc) as tc, ExitStack() as ctx:
        const_pool = ctx.enter_context(tc.tile_pool(name="const", bufs=1))
        w01_t = const_pool.tile([P, DIM], bf16)
        w2_t = const_pool.tile([D, DIM], bf16)
        qk_t = const_pool.tile([D, HPC * 2 * T], bf16)
        va_t = const_pool.tile([P, HPC * T], bf16)
        # q/k for all heads first on SP (the head-interleaved j=0 regions
        # need them almost immediately); eb via the Act DGE queue and va via
        # the Pool SWDGE queue so the three setup pipelines overlap
        for h in range(HPC):
            sl = bass.ds(h * 2 * T, 2 * T)
            nc.sync.dma_start(qk_t[:, sl], qk_d[:, sl])

        with (
            tc.tile_pool(name="eb", bufs=4) as eb_pool,
            tc.tile_pool(name="pexp", bufs=8) as pexp_pool,
            tc.tile_pool(name="rec", bufs=2) as rec_pool,
            tc.tile_pool(name="yt01", bufs=2) as yt01_pool,
            tc.tile_pool(name="yt2", bufs=2) as yt2_pool,
            tc.tile_pool(name="obig", bufs=2) as obig_pool,
            tc.tile_pool(name="psl", bufs=2, space="PSUM") as psl_pool,
            tc.tile_pool(name="psy", bufs=1, space="PSUM") as psy_pool,
            tc.tile_pool(name="psp", bufs=1, space="PSUM") as psp_pool,
        ):
            for j in range(NJ):
                yt01_t = yt01_pool.tile([P, QC], bf16)
                yt2_t = yt2_pool.tile([D, QC], bf16)
                ebw = _eb_cols(j)
                nreg = 2 * (j + 1)
                eb_ts, psy_ts, ebos, pvqs = [], [], [], []
                eb1_half = _eb_cols(1) // 2
                for h in range(HPC):
                    eb_t = eb_pool.tile([P, ebw], bf16)
                    if j == 1:
                        # first half was prefetched during j=0
                        eb_t1 = eb1_ts[h]
                        nc.sync.dma_start(
                            eb_t1[:, eb1_half:ebw],
                            eb_d[:, ds(h * EB_TOT + EB_OFF[1] + eb1_half,
                                       ebw - eb1_half)],
                        )
                    elif j == 0:
                        nc.gpsimd.dma_start(
                            eb_t[:], eb_d[:, ds(h * EB_TOT + EB_OFF[j], ebw)]
                        )
                    else:
                        nc.sync.dma_start(
                            eb_t[:], eb_d[:, ds(h * EB_TOT + EB_OFF[j], ebw)]
                        )
                    eb_ts.append(eb_t)
                    psy_t = psy_pool.tile([P, QC], f32, name="psy", tag=f"h{h}")
                    psy_ts.append(psy_t)
                    ebos.append(0)
                    pvqs.append([])
                if j == 1:
                    eb_ts = eb1_ts
                if j == 0:
                    for h in range(HPC):
                        sl = ds(h * T, T)
                        nc.gpsimd.dma_start(va_t[:, sl], va_d[:, sl])
                    # prefetch the first half of each eb(1,h) ahead of w
                    eb1_ts = []
                    for h in range(HPC):
                        eb1_t = eb_pool.tile(
                            [P, _eb_cols(1)], bf16, name="eb_t", tag="eb1"
                        )
                        nc.gpsimd.dma_start(
                            eb1_t[:, 0:eb1_half],
                            eb_d[:, ds(h * EB_TOT + EB_OFF[1], eb1_half)],
                        )
                        eb1_ts.append(eb1_t)
                    nc.sync.dma_start(w01_t[:], w01_d[:])
                    nc.sync.dma_start(w2_t[:], w2_d[:])

                def emit_pv(h, r, pe_t, cp):
                    c0s, pos = cp
                    for t in range(2):
                        i = 2 * r + t
                        c0 = c0s[t]
                        nc.tensor.matmul(
                            psy_ts[h][:, c0:QC],
                            lhsT=va_t[:, ds((h * NT + i) * P, P)],
                            rhs=pe_t[:, pos[t] : pos[t] + QC - c0],
                            start=(r == 0 and t == 0),
                            stop=(r == nreg - 1 and t == 1),
                        )

                # three independent head streams, round-robin by region
                for r in range(nreg):
                    for h in range(HPC):
                        is_diag = r >= 2 * j
                        psl_t = psl_pool.tile([P, 2 * QC], f32)
                        pe_t = pexp_pool.tile([P, 2 * QC], bf16)
                        c0s = []
                        # diag tiles pack their trimmed columns contiguously
                        # (matching the packed eb stream) so exp and the eb
                        # multiply each take ONE instruction per region
                        pos = []
                        po = 0
                        for t in range(2):
                            i = 2 * r + t
                            c0 = _c0(i, j) if is_diag else 0
                            c0s.append(c0)
                            pos.append(po)
                            nc.tensor.matmul(
                                psl_t[:, po : po + QC - c0],
                                lhsT=qk_t[
                                    :, ds(h * 2 * T + T + i * P, P)
                                ],
                                rhs=qk_t[
                                    :, ds(h * 2 * T + j * QC + c0, QC - c0)
                                ],
                                start=True,
                                stop=True,
                            )
                            po += QC - c0
                        ebo = ebos[h]
                        nc.scalar.activation(
                            pe_t[:, 0:po], psl_t[:, 0:po], EXP
                        )
                        meng = (
                            nc.gpsimd
                            if (not is_diag and (r * HPC + h) % 3 == 2)
                            else nc.vector
                        )
                        meng.tensor_mul(
                            pe_t[:, 0:po],
                            pe_t[:, 0:po],
                            eb_ts[h][:, ebo : ebo + po],
                        )
                        ebos[h] += po
                        pvqs[h].append((h, r, pe_t, (c0s, pos)))
                        if len(pvqs[h]) > 1:
                            emit_pv(*pvqs[h].pop(0))
                for h in range(HPC):
                    while pvqs[h]:
                        emit_pv(*pvqs[h].pop(0))
                    rec_t = rec_pool.tile([D, QC], f32)
                    nc.vector.reciprocal(rec_t[:], psy_ts[h][D : 2 * D, :])
                    if h == 0:
                        ydst = yt01_t[0:D, :]
                    elif h == 1:
                        ydst = yt01_t[D : 2 * D, :]
                    else:
                        ydst = yt2_t[:]
                    nc.vector.tensor_mul(ydst, psy_ts[h][0:D, :], rec_t[:])
                # projection for this q-chunk (4 t-blocks of 128):
                # 12 passes of 256, rotating over 4 banks (psp + the three
                # psy banks, idle between the last norm and the next chunk)
                o_t = obig_pool.tile([P, 4 * DIM], bf16)
                pidx = 0
                for tb in range(4):
                    for o0 in (0, 256, 512):
                        bank = pidx % 4
                        pidx += 1
                        if bank == 3:
                            psp_t = psp_pool.tile([P, 256], f32)
                        else:
                            psp_t = psy_pool.tile(
                                [P, 256], f32, name="psy", tag=f"h{bank}"
                            )
                        nc.tensor.matmul(
                            psp_t[:],
                            lhsT=yt01_t[:, tb * P : (tb + 1) * P],
                            rhs=w01_t[:, o0 : o0 + 256],
                            start=True,
                            stop=False,
                        )
                        nc.tensor.matmul(
                            psp_t[:],
                            lhsT=yt2_t[:, tb * P : (tb + 1) * P],
                            rhs=w2_t[:, o0 : o0 + 256],
                            start=False,
                            stop=True,
                        )
                        nc.vector.tensor_copy(
                            o_t[:, tb * DIM + o0 : tb * DIM + o0 + 256],
                            psp_t[:],
                        )
                    if j == NJ - 1:
                        nc.gpsimd.dma_start(
                            out_d[j * QC + tb * P : j * QC + (tb + 1) * P, :],
                            o_t[:, tb * DIM : (tb + 1) * DIM].rearrange(
                                "p (a o) -> p (a o)", a=1
                            ),
                        )
                    elif tb == 1:
                        nc.gpsimd.dma_start(
                            out_d[j * QC : j * QC + 2 * P, :].rearrange(
                                "(a p) o -> p a o", p=P
                            ),
                            o_t[:, 0 : 2 * DIM].rearrange(
                                "p (a o) -> p a o", a=2
                            ),
                        )
                if j != NJ - 1:
                    nc.gpsimd.dma_start(
                        out_d[j * QC + 2 * P : (j + 1) * QC, :].rearrange(
                            "(a p) o -> p a o", p=P
                        ),
                        o_t[:, 2 * DIM :].rearrange("p (a o) -> p a o", a=2),
                    )

    nc.finalize()
    return nc


def _get_program():
    global _PROGRAM
    if _PROGRAM is None:
        _PROGRAM = _build_program()
    return _PROGRAM


def make_in_maps(q, k, v, attn_bias, W_proj):
    """Host-side sharding/layout prep: one input map per core."""
    bf = ml_dtypes.bfloat16
    q = np.asarray(q, dtype=np.float32)
    k = np.asarray(k, dtype=np.float32)
    v = np.asarray(v, dtype=np.float32)
    attn_bias = np.asarray(attn_bias, dtype=np.float32)
    W_proj = np.asarray(W_proj, dtype=np.float32)

    scale = 1.0 / math.sqrt(D)
    w_heads = W_proj.reshape(H, D, DIM)
    # causal mask in [s, q] coords: valid where q >= s
    smask = np.arange(T)[:, None] <= np.arange(T)[None, :]

    in_maps = []
    for c in range(NCORES):
        bb = c // 4
        h0 = HPC * (c % 4)
        qk = np.empty((D, HPC, 2, T), dtype=bf)
        va = np.empty((P, HPC, NT, P), dtype=bf)
        eb = np.empty((P, HPC, EB_TOT), dtype=bf)
        for h in range(HPC):
            hh = h0 + h
            qk[:, h, 0, :] = (q[bb, hh].T * scale).astype(bf)
            qk[:, h, 1, :] = k[bb, hh].T.astype(bf)
            va[:, h, :, :D] = (
                v[bb, hh].reshape(NT, P, D).transpose(1, 0, 2).astype(bf)
            )
            va[:, h, :, D:] = 1.0
            ebf = np.exp(attn_bias[bb, hh].T) * smask  # [s, q]
            off = 0
            for j in range(NJ):
                for i in range(4 * j + 4):
                    c0 = _c0(i, j)
                    w = QC - c0
                    eb[:, h, off : off + w] = ebf[
                        i * P : (i + 1) * P, j * QC + c0 : (j + 1) * QC
                    ].astype(bf)
                    off += w
        in_maps.append(
            {
                "qk": qk.reshape(D, HPC * 2 * T),
                "va": va.reshape(P, HPC * T),
                "eb": eb.reshape(P, HPC * EB_TOT),
                "w01": np.ascontiguousarray(
                    w_heads[h0 : h0 + 2].reshape(P, DIM)
                ).astype(bf),
                "w2": w_heads[h0 + 2].astype(bf),
                "out": np.zeros((T, DIM), dtype=bf),
            }
        )
    return in_maps


def assemble_output(results):
    """Sum the 4 per-core partial projections for each batch."""
    out = np.zeros((B, T, DIM), dtype=np.float32)
    for c in range(NCORES):
        out[c // 4] += np.asarray(results[c]["out"], dtype=np.float32)
    return out


def kernel(q, k, v, attn_bias, W_proj):
    from concourse.bass_utils import run_bass_kernel_spmd

    nc = _get_program()
    in_maps = make_in_maps(q, k, v, attn_bias, W_proj)
    res = run_bass_kernel_spmd(nc, in_maps, list(range(NCORES)))
    return assemble_output(res.results)



# revision 18
# speedup vs baseline: 1.1421x; 1.1421x over previous
"""Causal attention + output projection on 8 Trainium2 NeuronCores, v6.

Problem (hardcoded): B=2, H=12, T=2048, D=64, DIM=768, fp32.

Sharding: 24 (b, h) pairs -> 3 heads per core; cores 0-3 take b=0,
cores 4-7 take b=1.  Each core computes attention for its 3 heads plus
the partial output projection; the host sums the 4 bf16 partials per
batch in fp32.  No collectives.

All matmuls run in bf16.  fp8 variants (DoubleRow at 0.5 cyc/row) were
measured to be off the error budget: independent per-element noise eps
on softmax weights or logits lands at ~sqrt(2)*eps relative-to-absmax
on the output (the 1/sqrt(Neff) attenuation cancels against the
output's own scale), so fp8's ~1.8% rms quantization alone costs
~2e-2.  The additive attention bias is applied MULTIPLICATIVELY:
the host ships eb = exp(bias) (0 where causally masked) in bf16 and
the device computes P = exp(qk) * eb -- exact masking for free.

Engine budget per core (Act is the wall; exp is scalar-engine-only at
0.833 ns/col + 185 ns/instr):
  Act  55.9us  exp only (60 instrs)
  PE   54.5us  QK 21.8 + PV 21.8 + proj 10.2 + overheads
  DVE  ~46us   eb-mul share (2x bf16 mode) + reciprocal + normalize
               + PSUM->SBUF proj copies (384-wide)
  Pool ~44us   eb-mul share + eb/va/out DMA (SWDGE)
  SP   ~43us   eb/qk/w/out DMA
Scheduling: three head streams round-robin per region; PV lags one
region; psl is a 2-deep [128,1024] PSUM ring; psy is one bank per
head; projection PSUM ping-pongs psp + the h2 psy bank and chunk j's
projection blocks + out DMAs are EMITTED interleaved into chunk j+1's
region stream (the engine queues are in-order, so emission order is
schedule order).  Pool-queue DMAs for chunk j+1 are likewise emitted
lazily during chunk j so they slot between Pool eb-muls.
"""

import math

import numpy as np
import ml_dtypes

B, H, T, D = 2, 12, 2048, 64
DIM = H * D
NCORES = 8
HPC = 3            # heads per core
P = 128
QC = 512           # q-chunk width
NJ = T // QC       # 4 q-chunks
NT = T // P        # 16 s-tiles

BF = ml_dtypes.bfloat16

_PROGRAM = None


def _c0(i, j):
    return max(0, P * i - QC * j)


def _regions(j):
    """[(r, wa, wb)] for chunk j: region r covers s-tiles 2r, 2r+1."""
    out = []
    for r in range(2 * (j + 1)):
        wa = QC - _c0(2 * r, j)
        wb = QC - _c0(2 * r + 1, j)
        out.append((r, wa, wb))
    return out


# eb packed offsets, (j, h)-major: chunk j's blocks for all heads are
# contiguous; within (j, h) the region blocks are r-major, each block
# [128 part, wa+wb] bf16 (tile-a cols then tile-b cols, causally trimmed).
EB_JCOLS = [sum(wa + wb for _, wa, wb in _regions(_j)) for _j in range(NJ)]
EB_JOFF = [0]
for _j in range(NJ):
    EB_JOFF.append(EB_JOFF[-1] + HPC * EB_JCOLS[_j])
EB_TOT = EB_JOFF[-1]   # 52224 cols per core


def _eb_off(j, h):
    return EB_JOFF[j] + h * EB_JCOLS[j]


def _build_program():
    import concourse.bass as bass
    import concourse.mybir as mybir
    import concourse.tile as tile
    from concourse import bacc
    from contextlib import ExitStack

    dt = mybir.dt
    f32 = dt.float32
    bf16 = dt.bfloat16
    EXP = mybir.ActivationFunctionType.Exp
    ds = bass.ds

    nc = bacc.Bacc("TRN2", num_devices=NCORES)
    k16_d = nc.declare_dram_parameter("k16", [D, HPC * T], bf16, isOutput=False)
    q16_d = nc.declare_dram_parameter("q16", [D, HPC * T], bf16, isOutput=False)
    eb_d = nc.declare_dram_parameter("eb", [P, EB_TOT], bf16,
                                     isOutput=False)
    va_d = nc.declare_dram_parameter("va", [P, HPC * NT * P], bf16,
                                     isOutput=False)
    w01_d = nc.declare_dram_parameter("w01", [P, DIM], bf16, isOutput=False)
    w2_d = nc.declare_dram_parameter("w2", [D, DIM], bf16, isOutput=False)
    out_d = nc.declare_dram_parameter("out", [T, DIM], bf16, isOutput=True)

    OB = 384  # projection o-block width (two blocks per 768-wide tb)

    with tile.TileContext(nc) as tc, ExitStack() as ctx:
        const_pool = ctx.enter_context(tc.tile_pool(name="const", bufs=1))
        k16_t = const_pool.tile([D, HPC * T], bf16)
        q16_t = const_pool.tile([D, HPC * T], bf16)
        eb_t = const_pool.tile([P, EB_TOT], bf16)
        va_t = const_pool.tile([P, HPC * NT * P], bf16)
        w01_t = const_pool.tile([P, DIM], bf16)
        w2_t = const_pool.tile([D, DIM], bf16)

        def eb_dma(eng, h, j, r0, r1):
            regs = _regions(j)
            o0 = _eb_off(j, h) + sum(wa + wb for _, wa, wb in regs[:r0])
            sz = sum(wa + wb for _, wa, wb in regs[r0:r1])
            eng.dma_start(eb_t[:, ds(o0, sz)], eb_d[:, ds(o0, sz)])

        # ---- startup DMAs, ordered by first-use deadline.  SP carries
        # most of the eb stream (it has no compute); Pool's DMAs all land
        # before chunk-2 so mid-stream Pool eb-muls never queue behind a
        # transfer.  Chunk 0/1 muls run on DVE only for the same reason.
        nc.sync.dma_start(k16_t[:, ds(0, QC)], k16_d[:, ds(0, QC)])
        nc.sync.dma_start(q16_t[:, ds(0, QC)], q16_d[:, ds(0, QC)])
        eb_dma(nc.gpsimd, 0, 0, 0, 2)
        nc.sync.dma_start(k16_t[:, ds(T, QC)], k16_d[:, ds(T, QC)])
        nc.sync.dma_start(q16_t[:, ds(T, QC)], q16_d[:, ds(T, QC)])
        eb_dma(nc.gpsimd, 1, 0, 0, 2)
        nc.sync.dma_start(k16_t[:, ds(2 * T, QC)], k16_d[:, ds(2 * T, QC)])
        nc.sync.dma_start(q16_t[:, ds(2 * T, QC)], q16_d[:, ds(2 * T, QC)])
        eb_dma(nc.gpsimd, 2, 0, 0, 2)
        nc.gpsimd.dma_start(va_t[:, ds(0, NT * P)], va_d[:, ds(0, NT * P)])
        # chunk-1 k/q + eb staggered across both queues per head
        nc.sync.dma_start(k16_t[:, ds(QC, T - QC)], k16_d[:, ds(QC, T - QC)])
        nc.sync.dma_start(q16_t[:, ds(QC, QC)], q16_d[:, ds(QC, QC)])
        eb_dma(nc.sync, 0, 1, 0, 2)
        nc.gpsimd.dma_start(va_t[:, ds(NT * P, NT * P)],
                            va_d[:, ds(NT * P, NT * P)])
        nc.sync.dma_start(k16_t[:, ds(T + QC, T - QC)],
                          k16_d[:, ds(T + QC, T - QC)])
        nc.sync.dma_start(q16_t[:, ds(T + QC, QC)], q16_d[:, ds(T + QC, QC)])
        eb_dma(nc.gpsimd, 2, 1, 0, 4)
        eb_dma(nc.sync, 0, 1, 2, 4)
        nc.sync.dma_start(k16_t[:, ds(2 * T + QC, T - QC)],
                          k16_d[:, ds(2 * T + QC, T - QC)])
        nc.sync.dma_start(q16_t[:, ds(2 * T + QC, QC)],
                          q16_d[:, ds(2 * T + QC, QC)])
        eb_dma(nc.sync, 1, 1, 0, 4)
        nc.gpsimd.dma_start(va_t[:, ds(2 * NT * P, NT * P)],
                            va_d[:, ds(2 * NT * P, NT * P)])
        eb_dma(nc.gpsimd, 2, 2, 0, 6)
        nc.sync.dma_start(q16_t[:, ds(2 * QC, QC)], q16_d[:, ds(2 * QC, QC)])
        nc.sync.dma_start(q16_t[:, ds(T + 2 * QC, QC)],
                          q16_d[:, ds(T + 2 * QC, QC)])
        nc.sync.dma_start(q16_t[:, ds(2 * T + 2 * QC, QC)],
                          q16_d[:, ds(2 * T + 2 * QC, QC)])
        nc.sync.dma_start(w01_t[:], w01_d[:])
        nc.sync.dma_start(w2_t[:], w2_d[:])
        eb_dma(nc.sync, 0, 2, 0, 6)
        eb_dma(nc.sync, 1, 2, 0, 6)
        nc.sync.dma_start(q16_t[:, ds(3 * QC, QC)], q16_d[:, ds(3 * QC, QC)])
        nc.sync.dma_start(q16_t[:, ds(T + 3 * QC, QC)],
                          q16_d[:, ds(T + 3 * QC, QC)])
        nc.sync.dma_start(q16_t[:, ds(2 * T + 3 * QC, QC)],
                          q16_d[:, ds(2 * T + 3 * QC, QC)])
        eb_dma(nc.gpsimd, 0, 3, 0, 4)
        eb_dma(nc.sync, 0, 3, 4, 8)
        eb_dma(nc.sync, 1, 3, 0, 8)
        eb_dma(nc.sync, 2, 3, 0, 8)

        def k_ap(h, i):
            return k16_t[:, ds(h * T + i * P, P)]

        def q_ap(h, j, c0):
            return q16_t[:, ds(h * T + j * QC + c0, QC - c0)]

        def va_ap(h, i):
            return va_t[:, ds((h * NT + i) * P, P)]

        with (
            tc.tile_pool(name="pexp", bufs=8) as pexp_pool,
            tc.tile_pool(name="rec", bufs=2) as rec_pool,
            tc.tile_pool(name="yt01", bufs=2) as yt01_pool,
            tc.tile_pool(name="yt2", bufs=2) as yt2_pool,
            tc.tile_pool(name="obig", bufs=4) as obig_pool,
            tc.tile_pool(name="psl", bufs=2, space="PSUM") as psl_pool,
            tc.tile_pool(name="psy", bufs=1, space="PSUM") as psy_pool,
            tc.tile_pool(name="psp", bufs=1, space="PSUM") as psp_pool,
        ):
            defer_q = []   # deferred thunks (proj blocks, out/eb DMAs)

            def drain(n):
                for _ in range(min(n, len(defer_q))):
                    defer_q.pop(0)()

            COPY = mybir.ActivationFunctionType.Copy

            def make_blk(tb, ob, bank, yt01_c, yt2_c, o_c, act_copy=False):
                # proj PSUM rotates over psp + the h2/h1 psy banks (their
                # next-chunk PVs are the last to need the banks back; lazy
                # psy allocation keeps buffer-reuse deps acyclic with the
                # in-order engine queues).  The last chunk also uses h0.
                def blk():
                    if bank == 0:
                        psp_t = psp_pool.tile([P, OB], f32)
                    else:
                        psp_t = psy_pool.tile([P, OB], f32, name="psy",
                                              tag=["h2", "h1", "h0"][bank - 1])
                    nc.tensor.matmul(
                        psp_t[:],
                        lhsT=yt01_c[:, tb * P:(tb + 1) * P],
                        rhs=w01_t[:, ds(ob * OB, OB)],
                        start=True, stop=False,
                    )
                    nc.tensor.matmul(
                        psp_t[:],
                        lhsT=yt2_c[:, tb * P:(tb + 1) * P],
                        rhs=w2_t[:, ds(ob * OB, OB)],
                        start=False, stop=True,
                    )
                    if act_copy:
                        nc.scalar.activation(
                            o_c[:, ds(tb * DIM + ob * OB, OB)], psp_t[:], COPY
                        )
                    else:
                        nc.vector.tensor_copy(
                            o_c[:, ds(tb * DIM + ob * OB, OB)], psp_t[:]
                        )
                return blk

            def make_out_dma(eng, jj, tb, o_c):
                def dma():
                    eng.dma_start(
                        out_d[jj * QC + tb * P:jj * QC + (tb + 1) * P, :],
                        o_c[:, ds(tb * DIM, DIM)],
                    )
                return dma

            mul_toggle = [0]

            for j in range(NJ):
                regs = _regions(j)
                nreg = len(regs)
                eoffs = [_eb_off(j, h) for h in range(HPC)]
                psy_ts = [None] * HPC
                pvqs = [[] for _ in range(HPC)]

                def get_psy(h, psy_ts=psy_ts):
                    if psy_ts[h] is None:
                        psy_ts[h] = psy_pool.tile([P, QC], f32, name="psy",
                                                  tag=f"h{h}")
                    return psy_ts[h]

                def emit_pv(h, r, pe_t, wa, wb, nreg=nreg,
                            get_psy=get_psy):
                    for t in range(2):
                        i = 2 * r + t
                        pos = 0 if t == 0 else wa
                        w = wa if t == 0 else wb
                        nc.tensor.matmul(
                            get_psy(h)[:, QC - w:QC],
                            lhsT=va_ap(h, i),
                            rhs=pe_t[:, ds(pos, w)],
                            start=(r == 0 and t == 0),
                            stop=(r == nreg - 1 and t == 1),
                        )

                if j == 0 or j == NJ - 1:
                    iter_order = [(r, wa, wb, h) for h in range(HPC)
                                  for r, wa, wb in regs]
                else:
                    iter_order = [(r, wa, wb, h) for r, wa, wb in regs
                                  for h in range(HPC)]
                yt01_t = yt01_pool.tile([P, QC], bf16)
                yt2_t = yt2_pool.tile([D, QC], bf16)

                def make_tailnorm(h, yt01_c, yt2_c, pvq, psy_c, epv):
                    def tail():
                        while pvq:
                            epv(*pvq.pop(0))
                        rec_t = rec_pool.tile([D, QC], f32)
                        nc.vector.reciprocal(rec_t[:], psy_c(h)[D:2 * D, :])
                        if h == 0:
                            ydst = yt01_c[0:D, :]
                        elif h == 1:
                            ydst = yt01_c[D:2 * D, :]
                        else:
                            ydst = yt2_c[:]
                        nc.vector.tensor_mul(ydst, psy_c(h)[0:D, :], rec_t[:])
                    return tail

                it = 0
                last = j == NJ - 1
                for r, wa, wb, h in iter_order:
                    if True:
                        po = wa + wb
                        psl_t = psl_pool.tile([P, 2 * QC], f32)
                        pe_t = pexp_pool.tile([P, 2 * QC], bf16)
                        nc.tensor.matmul(
                            psl_t[:, 0:wa],
                            lhsT=k_ap(h, 2 * r),
                            rhs=q_ap(h, j, QC - wa),
                            start=True, stop=True,
                        )
                        nc.tensor.matmul(
                            psl_t[:, wa:po],
                            lhsT=k_ap(h, 2 * r + 1),
                            rhs=q_ap(h, j, QC - wb),
                            start=True, stop=True,
                        )
                        nc.scalar.activation(
                            pe_t[:, 0:po], psl_t[:, 0:po], EXP
                        )
                        if j < 2:
                            meng = nc.vector
                        else:
                            meng = (nc.gpsimd if mul_toggle[0] % 5 < 3
                                    else nc.vector)
                            mul_toggle[0] += 1
                        meng.tensor_mul(
                            pe_t[:, 0:po],
                            pe_t[:, 0:po],
                            eb_t[:, ds(eoffs[h], po)],
                        )
                        eoffs[h] += po
                        pvqs[h].append((h, r, pe_t, wa, wb))
                        lag = 1 if j == 0 else (3 if last else 2)
                        if last and r >= nreg - 2:
                            while pvqs[h]:
                                emit_pv(*pvqs[h].pop(0))
                        elif len(pvqs[h]) > (lag if it < 6 else 1):
                            emit_pv(*pvqs[h].pop(0))
                        if last and r == nreg - 1:
                            # this head's stream is done; flush + norm now
                            # so only h2's norm remains after the last exp
                            defer_q.append(make_tailnorm(
                                h, yt01_t, yt2_t, pvqs[h], get_psy, emit_pv))
                        drain(0 if it < 2 else (3 if it < 5 else 2))
                        it += 1

                if not last:
                    for h in range(HPC):
                        defer_q.append(make_tailnorm(
                            h, yt01_t, yt2_t, pvqs[h], get_psy, emit_pv))

                o_t = obig_pool.tile([P, 4 * DIM], bf16)
                nbank = 4 if j == NJ - 1 else 3
                for tb in range(4):
                    for ob in range(2):
                        defer_q.append(make_blk(
                            tb, ob, (tb * 2 + ob) % nbank, yt01_t, yt2_t, o_t,
                            act_copy=(j == NJ - 1 and (tb * 2 + ob) % 2 == 1)))
                    defer_q.append(make_out_dma(
                        nc.sync if tb % 2 == 0 else nc.gpsimd, j, tb, o_t))

            drain(len(defer_q))

    nc.finalize()
    return nc


def _get_program():
    global _PROGRAM
    if _PROGRAM is None:
        _PROGRAM = _build_program()
    return _PROGRAM


def make_in_maps(q, k, v, attn_bias, W_proj):
    """Host-side sharding/layout prep: one input map per core."""
    q = np.asarray(q, dtype=np.float32)
    k = np.asarray(k, dtype=np.float32)
    v = np.asarray(v, dtype=np.float32)
    attn_bias = np.asarray(attn_bias, dtype=np.float32)
    W_proj = np.asarray(W_proj, dtype=np.float32)

    scale = 1.0 / math.sqrt(D)
    w_heads = W_proj.reshape(H, D, DIM)
    smask = np.arange(T)[:, None] <= np.arange(T)[None, :]  # [s, q] valid

    in_maps = []
    for c in range(NCORES):
        bb = c // 4
        h0 = HPC * (c % 4)
        k16 = np.empty((HPC, D, T), dtype=BF)
        q16 = np.empty((HPC, D, T), dtype=BF)
        eb = np.empty((P, EB_TOT), dtype=BF)
        va = np.zeros((P, HPC, NT, P), dtype=np.float32)
        for h in range(HPC):
            hh = h0 + h
            k16[h] = k[bb, hh].T.astype(BF)
            q16[h] = (q[bb, hh].T * scale).astype(BF)
            ebf = np.exp(attn_bias[bb, hh].T) * smask  # [s, q]
            for j in range(NJ):
                off = _eb_off(j, h)
                for r, wa, wb in _regions(j):
                    eb[:, off:off + wa] = ebf[
                        2 * r * P:(2 * r + 1) * P,
                        (j + 1) * QC - wa:(j + 1) * QC].astype(BF)
                    off += wa
                    eb[:, off:off + wb] = ebf[
                        (2 * r + 1) * P:(2 * r + 2) * P,
                        (j + 1) * QC - wb:(j + 1) * QC].astype(BF)
                    off += wb
            va[:, h, :, 0:D] = (
                v[bb, hh].reshape(NT, P, D).transpose(1, 0, 2))
            va[:, h, :, D:] = 1.0
        # k16/q16: [d, (h, t)] with heads contiguous per partition
        in_maps.append(
            {
                "k16": k16.transpose(1, 0, 2).reshape(D, HPC * T),
                "q16": q16.transpose(1, 0, 2).reshape(D, HPC * T),
                "eb": eb,
                "va": va.reshape(P, HPC * NT * P).astype(BF),
                "w01": np.ascontiguousarray(
                    w_heads[h0:h0 + 2].reshape(P, DIM)).astype(BF),
                "w2": w_heads[h0 + 2].astype(BF),
                "out": np.zeros((T, DIM), dtype=BF),
            }
        )
    return in_maps


def assemble_output(results):
    """Sum the 4 per-core partial projections for each batch."""
    out = np.zeros((B, T, DIM), dtype=np.float32)
    for c in range(NCORES):
        out[c // 4] += np.asarray(results[c]["out"], dtype=np.float32)
    return out


def kernel(q, k, v, attn_bias, W_proj):
    from concourse.bass_utils import run_bass_kernel_spmd

    nc = _get_program()
    in_maps = make_in_maps(q, k, v, attn_bias, W_proj)
    res = run_bass_kernel_spmd(nc, in_maps, list(range(NCORES)))
    return assemble_output(res.results)


# revision 34
# speedup vs baseline: 1.1983x; 1.0492x over previous
"""Causal attention + output projection on 8 Trainium2 NeuronCores, v6.

Problem (hardcoded): B=2, H=12, T=2048, D=64, DIM=768, fp32.

Sharding: 24 (b, h) pairs -> 3 heads per core; cores 0-3 take b=0,
cores 4-7 take b=1.  Each core computes attention for its 3 heads plus
the partial output projection; the host sums the 4 bf16 partials per
batch in fp32.  No collectives.

All matmuls run in bf16.  fp8 variants (DoubleRow at 0.5 cyc/row) were
measured to be off the error budget: independent per-element noise eps
on softmax weights or logits lands at ~sqrt(2)*eps relative-to-absmax
on the output (the 1/sqrt(Neff) attenuation cancels against the
output's own scale), so fp8's ~1.8% rms quantization alone costs
~2e-2.  The additive attention bias is applied MULTIPLICATIVELY:
the host ships eb = exp(bias) (0 where causally masked) in bf16 and
the device computes P = exp(qk) * eb -- exact masking for free.

Engine budget per core (Act is the wall; exp is scalar-engine-only at
0.833 ns/col + 185 ns/instr):
  Act  55.9us  exp only (60 instrs)
  PE   54.5us  QK 21.8 + PV 21.8 + proj 10.2 + overheads
  DVE  ~46us   eb-mul share (2x bf16 mode) + reciprocal + normalize
               + PSUM->SBUF proj copies (384-wide)
  Pool ~44us   eb-mul share + eb/va/out DMA (SWDGE)
  SP   ~43us   eb/qk/w/out DMA
Scheduling: three head streams round-robin per region; PV lags one
region; psl is a 2-deep [128,1024] PSUM ring; psy is one bank per
head; projection PSUM ping-pongs psp + the h2 psy bank and chunk j's
projection blocks + out DMAs are EMITTED interleaved into chunk j+1's
region stream (the engine queues are in-order, so emission order is
schedule order).  Pool-queue DMAs for chunk j+1 are likewise emitted
lazily during chunk j so they slot between Pool eb-muls.
"""

import math

import numpy as np
import ml_dtypes

B, H, T, D = 2, 12, 2048, 64
DIM = H * D
NCORES = 8
HPC = 3            # heads per core
P = 128
QC = 512           # q-chunk width
NJ = T // QC       # 4 q-chunks
NT = T // P        # 16 s-tiles

BF = ml_dtypes.bfloat16

_PROGRAM = None


def _c0(i, j):
    return max(0, P * i - QC * j)


def _regions(j):
    """[(r, wa, wb)] for chunk j: region r covers s-tiles 2r, 2r+1."""
    out = []
    for r in range(2 * (j + 1)):
        wa = QC - _c0(2 * r, j)
        wb = QC - _c0(2 * r + 1, j)
        out.append((r, wa, wb))
    return out


# eb packed offsets, (j, h)-major: chunk j's blocks for all heads are
# contiguous; within (j, h) the region blocks are r-major, each block
# [128 part, wa+wb] bf16 (tile-a cols then tile-b cols, causally trimmed).
EB_JCOLS = [sum(wa + wb for _, wa, wb in _regions(_j)) for _j in range(NJ)]
EB_JOFF = [0]
for _j in range(NJ):
    EB_JOFF.append(EB_JOFF[-1] + HPC * EB_JCOLS[_j])
EB_TOT = EB_JOFF[-1]   # 52224 cols per core


def _eb_off(j, h):
    return EB_JOFF[j] + h * EB_JCOLS[j]


def _build_program():
    import concourse.bass as bass
    import concourse.mybir as mybir
    import concourse.tile as tile
    from concourse import bacc
    from contextlib import ExitStack

    dt = mybir.dt
    f32 = dt.float32
    bf16 = dt.bfloat16
    EXP = mybir.ActivationFunctionType.Exp
    ds = bass.ds

    nc = bacc.Bacc("TRN2", num_devices=NCORES)
    k16_d = nc.declare_dram_parameter("k16", [D, HPC * T], bf16, isOutput=False)
    q16_d = nc.declare_dram_parameter("q16", [D, HPC * T], bf16, isOutput=False)
    eb_d = nc.declare_dram_parameter("eb", [P, EB_TOT], bf16,
                                     isOutput=False)
    va_d = nc.declare_dram_parameter("va", [P, HPC * NT * P], bf16,
                                     isOutput=False)
    w01_d = nc.declare_dram_parameter("w01", [P, DIM], bf16, isOutput=False)
    w2_d = nc.declare_dram_parameter("w2", [D, DIM], bf16, isOutput=False)
    out_d = nc.declare_dram_parameter("out", [T, DIM], bf16, isOutput=True)

    OB = 384  # projection o-block width (two blocks per 768-wide tb)

    with tile.TileContext(nc) as tc, ExitStack() as ctx:
        const_pool = ctx.enter_context(tc.tile_pool(name="const", bufs=1))
        k16_t = const_pool.tile([D, HPC * T], bf16)
        q16_t = const_pool.tile([D, HPC * T], bf16)
        eb_t = const_pool.tile([P, EB_TOT], bf16)
        va_t = const_pool.tile([P, HPC * NT * P], bf16)
        w01_t = const_pool.tile([P, DIM], bf16)
        w2_t = const_pool.tile([D, DIM], bf16)

        def eb_dma(eng, h, j, r0, r1):
            regs = _regions(j)
            o0 = _eb_off(j, h) + sum(wa + wb for _, wa, wb in regs[:r0])
            sz = sum(wa + wb for _, wa, wb in regs[r0:r1])
            eng.dma_start(eb_t[:, ds(o0, sz)], eb_d[:, ds(o0, sz)])

        # ---- startup DMAs, ordered by first-use deadline.  SP carries
        # most of the eb stream (it has no compute); Pool's DMAs all land
        # before chunk-2 so mid-stream Pool eb-muls never queue behind a
        # transfer.  Chunk 0/1 muls run on DVE only for the same reason.
        nc.sync.dma_start(k16_t[:, ds(0, QC)], k16_d[:, ds(0, QC)])
        nc.sync.dma_start(q16_t[:, ds(0, QC)], q16_d[:, ds(0, QC)])
        eb_dma(nc.gpsimd, 0, 0, 0, 2)
        nc.sync.dma_start(k16_t[:, ds(T, QC)], k16_d[:, ds(T, QC)])
        nc.sync.dma_start(q16_t[:, ds(T, QC)], q16_d[:, ds(T, QC)])
        eb_dma(nc.gpsimd, 1, 0, 0, 2)
        nc.sync.dma_start(k16_t[:, ds(2 * T, QC)], k16_d[:, ds(2 * T, QC)])
        nc.sync.dma_start(q16_t[:, ds(2 * T, QC)], q16_d[:, ds(2 * T, QC)])
        eb_dma(nc.gpsimd, 2, 0, 0, 2)
        nc.gpsimd.dma_start(va_t[:, ds(0, NT * P)], va_d[:, ds(0, NT * P)])
        # chunk-1 q for all heads before the k tails (their first use is
        # earlier); k tails cover s-tiles 4..15 which only chunk-1 r2+ needs
        nc.sync.dma_start(q16_t[:, ds(QC, QC)], q16_d[:, ds(QC, QC)])
        nc.sync.dma_start(q16_t[:, ds(T + QC, QC)], q16_d[:, ds(T + QC, QC)])
        nc.sync.dma_start(q16_t[:, ds(2 * T + QC, QC)],
                          q16_d[:, ds(2 * T + QC, QC)])
        nc.sync.dma_start(w01_t[:], w01_d[:])
        nc.sync.dma_start(w2_t[:], w2_d[:])
        nc.sync.dma_start(k16_t[:, ds(QC, T - QC)], k16_d[:, ds(QC, T - QC)])
        eb_dma(nc.sync, 0, 1, 0, 2)
        nc.gpsimd.dma_start(va_t[:, ds(NT * P, NT * P)],
                            va_d[:, ds(NT * P, NT * P)])
        nc.sync.dma_start(k16_t[:, ds(T + QC, T - QC)],
                          k16_d[:, ds(T + QC, T - QC)])
        eb_dma(nc.gpsimd, 2, 1, 0, 4)
        eb_dma(nc.sync, 0, 1, 2, 4)
        nc.sync.dma_start(k16_t[:, ds(2 * T + QC, T - QC)],
                          k16_d[:, ds(2 * T + QC, T - QC)])
        eb_dma(nc.sync, 1, 1, 0, 4)
        nc.gpsimd.dma_start(va_t[:, ds(2 * NT * P, NT * P)],
                            va_d[:, ds(2 * NT * P, NT * P)])
        eb_dma(nc.gpsimd, 0, 2, 0, 3)
        eb_dma(nc.gpsimd, 2, 2, 0, 6)
        nc.sync.dma_start(q16_t[:, ds(2 * QC, QC)], q16_d[:, ds(2 * QC, QC)])
        nc.sync.dma_start(q16_t[:, ds(T + 2 * QC, QC)],
                          q16_d[:, ds(T + 2 * QC, QC)])
        nc.sync.dma_start(q16_t[:, ds(2 * T + 2 * QC, QC)],
                          q16_d[:, ds(2 * T + 2 * QC, QC)])
        eb_dma(nc.sync, 0, 2, 3, 6)
        eb_dma(nc.sync, 1, 2, 0, 6)
        nc.sync.dma_start(q16_t[:, ds(3 * QC, QC)], q16_d[:, ds(3 * QC, QC)])
        nc.sync.dma_start(q16_t[:, ds(T + 3 * QC, QC)],
                          q16_d[:, ds(T + 3 * QC, QC)])
        nc.sync.dma_start(q16_t[:, ds(2 * T + 3 * QC, QC)],
                          q16_d[:, ds(2 * T + 3 * QC, QC)])
        eb_dma(nc.gpsimd, 0, 3, 0, 4)
        eb_dma(nc.sync, 0, 3, 4, 8)
        eb_dma(nc.sync, 1, 3, 0, 8)
        eb_dma(nc.sync, 2, 3, 0, 8)

        def k_ap(h, i):
            return k16_t[:, ds(h * T + i * P, P)]

        def q_ap(h, j, c0):
            return q16_t[:, ds(h * T + j * QC + c0, QC - c0)]

        def va_ap(h, i):
            return va_t[:, ds((h * NT + i) * P, P)]

        with (
            tc.tile_pool(name="pexp", bufs=8) as pexp_pool,
            tc.tile_pool(name="rec", bufs=2) as rec_pool,
            tc.tile_pool(name="yt01", bufs=2) as yt01_pool,
            tc.tile_pool(name="yt2", bufs=2) as yt2_pool,
            tc.tile_pool(name="obig", bufs=4) as obig_pool,
            tc.tile_pool(name="psl", bufs=2, space="PSUM") as psl_pool,
            tc.tile_pool(name="psy", bufs=1, space="PSUM") as psy_pool,
            tc.tile_pool(name="psp", bufs=1, space="PSUM") as psp_pool,
        ):
            defer_q = []   # deferred thunks (proj blocks, out/eb DMAs)

            def drain(n):
                for _ in range(min(n, len(defer_q))):
                    defer_q.pop(0)()

            COPY = mybir.ActivationFunctionType.Copy

            def make_blk(tb, ob, bank, yt01_c, yt2_c, o_c, act_copy=False):
                # proj PSUM rotates over psp + the h2/h1 psy banks (their
                # next-chunk PVs are the last to need the banks back; lazy
                # psy allocation keeps buffer-reuse deps acyclic with the
                # in-order engine queues).  The last chunk also uses h0.
                def blk():
                    if bank == 0:
                        psp_t = psp_pool.tile([P, OB], f32)
                    else:
                        psp_t = psy_pool.tile([P, OB], f32, name="psy",
                                              tag=["h2", "h1", "h0"][bank - 1])
                    nc.tensor.matmul(
                        psp_t[:],
                        lhsT=yt01_c[:, tb * P:(tb + 1) * P],
                        rhs=w01_t[:, ds(ob * OB, OB)],
                        start=True, stop=False,
                    )
                    nc.tensor.matmul(
                        psp_t[:],
                        lhsT=yt2_c[:, tb * P:(tb + 1) * P],
                        rhs=w2_t[:, ds(ob * OB, OB)],
                        start=False, stop=True,
                    )
                    if act_copy:
                        nc.scalar.activation(
                            o_c[:, ds(tb * DIM + ob * OB, OB)], psp_t[:], COPY
                        )
                    else:
                        nc.vector.tensor_copy(
                            o_c[:, ds(tb * DIM + ob * OB, OB)], psp_t[:]
                        )
                return blk

            def make_out_dma(eng, jj, tb, o_c):
                def dma():
                    eng.dma_start(
                        out_d[jj * QC + tb * P:jj * QC + (tb + 1) * P, :],
                        o_c[:, ds(tb * DIM, DIM)],
                    )
                return dma

            mul_toggle = [0]

            for j in range(NJ):
                regs = _regions(j)
                nreg = len(regs)
                eoffs = [_eb_off(j, h) for h in range(HPC)]
                psy_ts = [None] * HPC
                pvqs = [[] for _ in range(HPC)]

                def get_psy(h, psy_ts=psy_ts):
                    if psy_ts[h] is None:
                        psy_ts[h] = psy_pool.tile([P, QC], f32, name="psy",
                                                  tag=f"h{h}")
                    return psy_ts[h]

                def emit_pv(h, r, pe_t, wa, wb, nreg=nreg,
                            get_psy=get_psy):
                    for t in range(2):
                        i = 2 * r + t
                        pos = 0 if t == 0 else wa
                        w = wa if t == 0 else wb
                        nc.tensor.matmul(
                            get_psy(h)[:, QC - w:QC],
                            lhsT=va_ap(h, i),
                            rhs=pe_t[:, ds(pos, w)],
                            start=(r == 0 and t == 0),
                            stop=(r == nreg - 1 and t == 1),
                        )

                if j == 0 or j == NJ - 1:
                    iter_order = [(r, wa, wb, h) for h in range(HPC)
                                  for r, wa, wb in regs]
                else:
                    iter_order = [(r, wa, wb, h) for r, wa, wb in regs
                                  for h in range(HPC)]
                yt01_t = yt01_pool.tile([P, QC], bf16)
                yt2_t = yt2_pool.tile([D, QC], bf16)

                def make_tailnorm(h, yt01_c, yt2_c, pvq, psy_c, epv,
                                  split=False):
                    def tail():
                        while pvq:
                            epv(*pvq.pop(0))
                        rec_t = rec_pool.tile([D, QC], f32)
                        nc.vector.reciprocal(rec_t[:], psy_c(h)[D:2 * D, :])
                        if h == 0:
                            ydst = yt01_c[0:D, :]
                        elif h == 1:
                            ydst = yt01_c[D:2 * D, :]
                        else:
                            ydst = yt2_c[:]
                        if split:
                            for tb in range(4):
                                nc.vector.tensor_mul(
                                    ydst[:, tb * P:(tb + 1) * P],
                                    psy_c(h)[0:D, tb * P:(tb + 1) * P],
                                    rec_t[:, tb * P:(tb + 1) * P])
                        else:
                            nc.vector.tensor_mul(ydst, psy_c(h)[0:D, :],
                                                 rec_t[:])
                    return tail

                it = 0
                last = j == NJ - 1
                for r, wa, wb, h in iter_order:
                    if True:
                        po = wa + wb
                        psl_t = psl_pool.tile([P, 2 * QC], f32)
                        pe_t = pexp_pool.tile([P, 2 * QC], bf16)
                        nc.tensor.matmul(
                            psl_t[:, 0:wa],
                            lhsT=k_ap(h, 2 * r),
                            rhs=q_ap(h, j, QC - wa),
                            start=True, stop=True,
                        )
                        nc.tensor.matmul(
                            psl_t[:, wa:po],
                            lhsT=k_ap(h, 2 * r + 1),
                            rhs=q_ap(h, j, QC - wb),
                            start=True, stop=True,
                        )
                        nc.scalar.activation(
                            pe_t[:, 0:po], psl_t[:, 0:po], EXP
                        )
                        if j < 2:
                            meng = nc.vector
                        else:
                            meng = (nc.gpsimd if mul_toggle[0] % 5 < 3
                                    else nc.vector)
                            mul_toggle[0] += 1
                        meng.tensor_mul(
                            pe_t[:, 0:po],
                            pe_t[:, 0:po],
                            eb_t[:, ds(eoffs[h], po)],
                        )
                        eoffs[h] += po
                        pvqs[h].append((h, r, pe_t, wa, wb))
                        lag = 1 if j == 0 else (3 if last else 2)
                        if last and r >= nreg - 2:
                            while pvqs[h]:
                                emit_pv(*pvqs[h].pop(0))
                        elif len(pvqs[h]) > (lag if it < 6 else 1):
                            emit_pv(*pvqs[h].pop(0))
                        if last and r == nreg - 1:
                            # this head's stream is done; flush + norm now
                            # so only h2's norm remains after the last exp
                            defer_q.append(make_tailnorm(
                                h, yt01_t, yt2_t, pvqs[h], get_psy, emit_pv,
                                split=False))
                        if it >= 2:
                            drain(1)
                        it += 1

                if not last:
                    # tails go to the queue FRONT: they must drain before
                    # the next chunk's first lazy-psy PV allocation
                    for h in reversed(range(HPC)):
                        defer_q.insert(0, make_tailnorm(
                            h, yt01_t, yt2_t, pvqs[h], get_psy, emit_pv))

                o_t = obig_pool.tile([P, 4 * DIM], bf16)
                for tb in range(4):
                    for ob in range(2):
                        # mid-stream proj lives on psp only (no psy-tag
                        # ordering constraints); the last chunk's burst
                        # rotates all 4 banks and splits copies Act/DVE
                        defer_q.append(make_blk(
                            tb, ob, (tb * 2 + ob) % 4 if last else 0,
                            yt01_t, yt2_t, o_t,
                            act_copy=(last and (tb * 2 + ob) % 2 == 1)))
                    defer_q.append(make_out_dma(
                        nc.sync if tb % 2 == 0 else nc.gpsimd, j, tb, o_t))

            drain(len(defer_q))

    nc.finalize()
    return nc


def _get_program():
    global _PROGRAM
    if _PROGRAM is None:
        _PROGRAM = _build_program()
    return _PROGRAM


def make_in_maps(q, k, v, attn_bias, W_proj):
    """Host-side sharding/layout prep: one input map per core."""
    q = np.asarray(q, dtype=np.float32)
    k = np.asarray(k, dtype=np.float32)
    v = np.asarray(v, dtype=np.float32)
    attn_bias = np.asarray(attn_bias, dtype=np.float32)
    W_proj = np.asarray(W_proj, dtype=np.float32)

    scale = 1.0 / math.sqrt(D)
    w_heads = W_proj.reshape(H, D, DIM)
    smask = np.arange(T)[:, None] <= np.arange(T)[None, :]  # [s, q] valid

    in_maps = []
    for c in range(NCORES):
        bb = c // 4
        h0 = HPC * (c % 4)
        k16 = np.empty((HPC, D, T), dtype=BF)
        q16 = np.empty((HPC, D, T), dtype=BF)
        eb = np.empty((P, EB_TOT), dtype=BF)
        va = np.zeros((P, HPC, NT, P), dtype=np.float32)
        for h in range(HPC):
            hh = h0 + h
            k16[h] = k[bb, hh].T.astype(BF)
            q16[h] = (q[bb, hh].T * scale).astype(BF)
            ebf = np.exp(attn_bias[bb, hh].T) * smask  # [s, q]
            for j in range(NJ):
                off = _eb_off(j, h)
                for r, wa, wb in _regions(j):
                    eb[:, off:off + wa] = ebf[
                        2 * r * P:(2 * r + 1) * P,
                        (j + 1) * QC - wa:(j + 1) * QC].astype(BF)
                    off += wa
                    eb[:, off:off + wb] = ebf[
                        (2 * r + 1) * P:(2 * r + 2) * P,
                        (j + 1) * QC - wb:(j + 1) * QC].astype(BF)
                    off += wb
            va[:, h, :, 0:D] = (
                v[bb, hh].reshape(NT, P, D).transpose(1, 0, 2))
            va[:, h, :, D:] = 1.0
        # k16/q16: [d, (h, t)] with heads contiguous per partition
        in_maps.append(
            {
                "k16": k16.transpose(1, 0, 2).reshape(D, HPC * T),
                "q16": q16.transpose(1, 0, 2).reshape(D, HPC * T),
                "eb": eb,
                "va": va.reshape(P, HPC * NT * P).astype(BF),
                "w01": np.ascontiguousarray(
                    w_heads[h0:h0 + 2].reshape(P, DIM)).astype(BF),
                "w2": w_heads[h0 + 2].astype(BF),
                "out": np.zeros((T, DIM), dtype=BF),
            }
        )
    return in_maps


def assemble_output(results):
    """Sum the 4 per-core partial projections for each batch."""
    out = np.zeros((B, T, DIM), dtype=np.float32)
    for c in range(NCORES):
        out[c // 4] += np.asarray(results[c]["out"], dtype=np.float32)
    return out


def kernel(q, k, v, attn_bias, W_proj):
    from concourse.bass_utils import run_bass_kernel_spmd

    nc = _get_program()
    in_maps = make_in_maps(q, k, v, attn_bias, W_proj)
    res = run_bass_kernel_spmd(nc, in_maps, list(range(NCORES)))
    return assemble_output(res.results)
